# revision 1
# baseline (speedup 1.0000x reference)
"""GCNII (nn_GCNII_17626545783193) Bass/Trainium2 kernel, 8 NeuronCores.

Strategy (target-node sharding, feature-major compute):
  - Nodes sharded 12500/core (padded to 12544 = 98*128). Edges partitioned
    by target core; self-loops included as real edges.
  - gcn_norm factored: norm[e] = dinv[src]*dinv[dst]. The dinv[src] factor is
    folded into the gathered features (hs = dinv * h, replicated via AllGather
    each layer); 0.9*dinv[dst] is folded into the one-hot scatter matrix.
  - Per layer, per core: dma_gather of in-edge source rows (256B each) from
    hs_full; DVE builds one-hot [128edge, 128target] tiles (iota==tloc)*dcol;
    PE computes agg^T = msgs^T @ onehot accumulated in PSUM [64, 512];
    support = agg + 0.1*h0^T; GCNII dense update via two accumulating
    matmuls (beta*Wl and (1-beta)*I, host-prescaled); ACT relu; PE transpose
    back to node-major; scale by dinv; AllGather.
"""
import sys
sys.path.insert(0, "/opt/trn_rl_repo")

import math
import numpy as np

# ---- problem constants (hardcoded per spec) ----
N_NODES = 100000
N_FEATS = 128
HIDDEN = 64
N_CLASSES = 40
N_LAYERS = 8
ALPHA = 0.1
LAMDA = 0.5
NCORES = 8

NPC = N_NODES // NCORES            # 12500 real nodes per core
TPC = (NPC + 127) // 128           # 98 target tiles per core
NPC_PAD = TPC * 128                # 12544
N_PAD = NPC_PAD * NCORES           # 100352
GROUP_TT = 8                       # target tiles per gather group
SUB_TT = 4                         # target tiles per psum tile (512 cols)
SRC_CHUNK = 32768                  # int16 index range per gather source chunk
NCHUNKS = (N_PAD + SRC_CHUNK - 1) // SRC_CHUNK  # 4
CALL_MAX = 8192                    # max indices per dma_gather call

TRACE = False          # test.py sets this for profiling
_LAST_RESULT = {}      # test.py reads exec_time from here


def _preprocess(edge_index):
    """Build the uniform (cross-core identical) edge stream structure and the
    per-core data arrays."""
    row = np.asarray(edge_index[0], dtype=np.int64)
    col = np.asarray(edge_index[1], dtype=np.int64)
    deg = np.bincount(col, minlength=N_NODES).astype(np.float32) + 1.0
    dinv = (1.0 / np.sqrt(deg)).astype(np.float32)

    loops = np.arange(N_NODES, dtype=np.int64)
    r = np.concatenate([row, loops])
    cl = np.concatenate([col, loops])
    core = cl // NPC
    lcol = cl - core * NPC                       # 0..12499 local target
    tt = lcol >> 7                               # target tile 0..97
    gp = (r // NPC) * NPC_PAD + (r % NPC)        # padded global source id
    k = gp // SRC_CHUNK                          # source chunk 0..3

    ngroups = (TPC + GROUP_TT - 1) // GROUP_TT

    slot = (core * TPC + tt) * NCHUNKS + k
    counts = np.bincount(slot, minlength=NCORES * TPC * NCHUNKS).reshape(
        NCORES, TPC, NCHUNKS
    )
    maxc = counts.max(axis=0)                    # [TPC, NCHUNKS]
    slot_chunks = -(-maxc // 128)                # ceil; 0 allowed
    slot_len = slot_chunks * 128

    slot_off = np.zeros((TPC, NCHUNKS), np.int64)
    off = 0
    chunk_tt = []
    calls_by_group = [[] for _ in range(ngroups)]
    for g in range(ngroups):
        tts = range(g * GROUP_TT, min((g + 1) * GROUP_TT, TPC))
        for kk in range(NCHUNKS):
            run_start = off
            for t in tts:
                slot_off[t, kk] = off
                off += slot_len[t, kk]
                chunk_tt += [t] * slot_chunks[t, kk]
            s = run_start
            while s < off:
                n = min(CALL_MAX, off - s)
                calls_by_group[g].append((kk, int(s), int(n)))
                s += n
    e_pad = off
    nchk = e_pad // 128
    chunk_tt = np.asarray(chunk_tt, np.int64)
    # start/stop are per PSUM tile (= subgroup of SUB_TT ttiles): exactly one
    # start=True (chronologically first matmul into the bank) and one
    # stop=True (last). A second start into a live bank wipes it (bank-wide
    # pending-zero), so interleaved per-ttile groups are NOT allowed.
    sub_id = chunk_tt // SUB_TT
    first = np.zeros(nchk, bool)
    last = np.zeros(nchk, bool)
    for sg in np.unique(sub_id):
        js = np.nonzero(sub_id == sg)[0]
        first[js[0]] = True
        last[js[-1]] = True

    per_core = []
    for c in range(NCORES):
        m = core == c
        rc = r[m]
        clm = cl[m]
        lc = lcol[m]
        tc_ = tt[m]
        kc = k[m]
        gpc = gp[m]
        skey = tc_ * NCHUNKS + kc
        order = np.argsort(skey, kind="stable")
        sorted_key = skey[order]
        firsts = np.searchsorted(sorted_key, sorted_key, side="left")
        ranks = np.arange(len(sorted_key)) - firsts
        pos = slot_off[tc_[order], kc[order]] + ranks

        idx_stream = np.zeros(e_pad, np.int16)
        tloc_stream = np.zeros(e_pad, np.float32)
        dcol_stream = np.zeros(e_pad, np.float32)
        idx_stream[pos] = (gpc[order] - kc[order] * SRC_CHUNK).astype(np.int16)
        tloc_stream[pos] = (lc[order] & 127).astype(np.float32)
        dcol_stream[pos] = ((1.0 - ALPHA) * dinv[clm[order]]).astype(np.float32)

        idx_w = np.zeros((128, e_pad // 16), np.int16)
        for g in range(ngroups):
            for (kk, s, n) in calls_by_group[g]:
                blk = idx_stream[s:s + n].reshape(n // 16, 16).T
                idx_w[:, s // 16:(s + n) // 16] = np.tile(blk, (8, 1))
        tloc_arr = np.ascontiguousarray(tloc_stream.reshape(nchk, 128).T)
        dcol_arr = np.ascontiguousarray(dcol_stream.reshape(nchk, 128).T)
        dv = np.zeros(NPC_PAD, np.float32)
        dv[:NPC] = dinv[c * NPC:(c + 1) * NPC]
        dinvo = np.ascontiguousarray(dv.reshape(TPC, 128).T)
        per_core.append(
            dict(idx=idx_w, tloc=tloc_arr, dcol=dcol_arr, dinvo=dinvo)
        )

    struct = dict(
        e_pad=e_pad,
        nchk=nchk,
        ngroups=ngroups,
        calls_by_group=calls_by_group,
        chunk_tt=chunk_tt,
        first=first,
        last=last,
    )
    return struct, per_core


def _build_program(struct):
    import concourse.bass as bass
    import concourse.mybir as mybir
    import concourse.tile as tile
    import concourse.bacc as bacc

    dt = mybir.dt
    f32 = dt.float32
    Alu = mybir.AluOpType
    Act = mybir.ActivationFunctionType

    e_pad = struct["e_pad"]
    nchk = struct["nchk"]
    ngroups = struct["ngroups"]
    calls_by_group = struct["calls_by_group"]
    chunk_tt = struct["chunk_tt"]
    first = struct["first"]
    last = struct["last"]
    rg = [list(range(NCORES))]

    nc = bacc.Bacc("TRN2", target_bir_lowering=False, debug=False,
                   num_devices=NCORES)

    x_in = nc.dram_tensor("x", [NPC_PAD, N_FEATS], f32, kind="ExternalInput")
    idx_in = nc.dram_tensor("idx", [128, e_pad // 16], dt.int16,
                            kind="ExternalInput")
    tloc_in = nc.dram_tensor("tloc", [128, nchk], f32, kind="ExternalInput")
    dcol_in = nc.dram_tensor("dcol", [128, nchk], f32, kind="ExternalInput")
    dinvo_in = nc.dram_tensor("dinvo", [128, TPC], f32, kind="ExternalInput")
    w0_in = nc.dram_tensor("w0", [N_FEATS, HIDDEN], f32, kind="ExternalInput")
    b0_in = nc.dram_tensor("b0", [HIDDEN, 1], f32, kind="ExternalInput")
    wl_in = nc.dram_tensor("wl", [HIDDEN, N_LAYERS * HIDDEN], f32,
                           kind="ExternalInput")
    il_in = nc.dram_tensor("il", [HIDDEN, N_LAYERS * HIDDEN], f32,
                           kind="ExternalInput")
    wout_in = nc.dram_tensor("wout", [HIDDEN, N_CLASSES], f32,
                             kind="ExternalInput")
    bout_in = nc.dram_tensor("bout", [N_CLASSES, 1], f32, kind="ExternalInput")
    iota_in = nc.dram_tensor("iota", [128, 128], f32, kind="ExternalInput")
    ident_in = nc.dram_tensor("ident", [128, 128], f32, kind="ExternalInput")
    out_dram = nc.dram_tensor("out", [NPC_PAD, N_CLASSES], f32,
                              kind="ExternalOutput")

    with tile.TileContext(nc) as tc:
        with (
            tc.tile_pool(name="persist", bufs=1) as P,
            tc.tile_pool(name="work", bufs=3) as S,
            tc.tile_pool(name="msgp", bufs=3) as MSG,
            tc.tile_pool(name="ohp", bufs=6) as OH,
            tc.tile_pool(name="psagg", bufs=4, space="PSUM") as PSA,
            tc.tile_pool(name="psmisc", bufs=2, space="PSUM") as PSM,
            tc.tile_pool(name="dram", bufs=1, space="DRAM") as DR,
        ):
            def pload(name, src, shape, dtype=f32):
                t = P.tile(shape, dtype, tag=name)
                nc.sync.dma_start(t[:], src[:])
                return t

            iota = pload("iota", iota_in, [128, 128])
            ident = pload("ident", ident_in, [128, 128])
            w0 = pload("w0", w0_in, [N_FEATS, HIDDEN])
            b0 = pload("b0", b0_in, [HIDDEN, 1])
            wl = pload("wl", wl_in, [HIDDEN, N_LAYERS * HIDDEN])
            il = pload("il", il_in, [HIDDEN, N_LAYERS * HIDDEN])
            wout = pload("wout", wout_in, [HIDDEN, N_CLASSES])
            bout = pload("bout", bout_in, [N_CLASSES, 1])
            tloc = pload("tloc", tloc_in, [128, nchk])
            dcol = pload("dcol", dcol_in, [128, nchk])
            dinvo = pload("dinvo", dinvo_in, [128, TPC])
            h0s = P.tile([HIDDEN, NPC_PAD], f32, tag="h0s")

            hs_shard = [
                DR.tile([NPC_PAD, HIDDEN], f32, tag=f"shard{j}",
                        name=f"hs_shard{j}")
                for j in range(N_LAYERS)
            ]
            hs_full = [
                DR.tile([N_PAD, HIDDEN], f32, tag=f"full{j}",
                        addr_space="Shared", name=f"hs_full{j}")
                for j in range(N_LAYERS)
            ]

            # ---------------- layer 0: h0 = relu(x @ W0 + b0) ----------------
            for t in range(TPC):
                xt = S.tile([128, N_FEATS], f32, tag="xt")
                nc.sync.dma_start(xt[:], x_in[t * 128:(t + 1) * 128, :])
                xT_ps = PSM.tile([N_FEATS, 128], f32, tag="tp")
                nc.tensor.transpose(xT_ps[:], xt[:], ident[:])
                xT = S.tile([N_FEATS, 128], f32, tag="xT")
                nc.vector.tensor_copy(xT[:], xT_ps[:])
                h_ps = PSM.tile([HIDDEN, 128], f32, tag="dense")
                nc.tensor.matmul(h_ps[:], lhsT=w0[:], rhs=xT[:],
                                 start=True, stop=True)
                h0t = S.tile([HIDDEN, 128], f32, tag="h0t")
                nc.scalar.activation(h0t[:], h_ps[:], Act.Relu, bias=b0[:])
                nc.vector.tensor_scalar_mul(
                    h0s[:, t * 128:(t + 1) * 128], h0t[:], ALPHA)
                tp_ps = PSM.tile([128, HIDDEN], f32, tag="tp")
                nc.tensor.transpose(tp_ps[:], h0t[:],
                                    ident[:HIDDEN, :HIDDEN])
                hs_sb = S.tile([128, HIDDEN], f32, tag="hs")
                nc.vector.tensor_scalar(hs_sb[:], tp_ps[:],
                                        dinvo[:, t:t + 1], None,
                                        op0=Alu.mult)
                nc.sync.dma_start(
                    hs_shard[0][t * 128:(t + 1) * 128, :], hs_sb[:])
            nc.gpsimd.collective_compute(
                "AllGather", Alu.bypass, replica_groups=rg,
                ins=[hs_shard[0].opt()], outs=[hs_full[0].opt()])

            # ---------------- GCNII layers ----------------
            for li in range(N_LAYERS):
                cur = hs_full[li]
                is_last = li == N_LAYERS - 1
                nxt_shard = hs_shard[li + 1] if not is_last else None
                for g in range(ngroups):
                    tts = list(range(g * GROUP_TT,
                                     min((g + 1) * GROUP_TT, TPC)))
                    subs = [tts[i:i + SUB_TT]
                            for i in range(0, len(tts), SUB_TT)]
                    ps_tiles = [PSA.tile([HIDDEN, SUB_TT * 128], f32,
                                         tag="agg", name=f"agg{li}_{g}_{si}")
                                for si in range(len(subs))]
                    for (kk, s, n) in calls_by_group[g]:
                        idx_t = S.tile([128, n // 16], dt.int16, tag="idxs")
                        nc.sync.dma_start(
                            idx_t[:], idx_in[:, s // 16:(s + n) // 16])
                        msg = MSG.tile([128, n // 128, HIDDEN], f32,
                                       tag="msg")
                        rows_k = min(SRC_CHUNK, N_PAD - kk * SRC_CHUNK)
                        nc.gpsimd.dma_gather(
                            msg[:],
                            cur[kk * SRC_CHUNK:kk * SRC_CHUNK + rows_k, :],
                            idx_t[:], num_idxs=n, num_idxs_reg=n,
                            elem_size=HIDDEN, single_packet=False)
                        for jj in range(n // 128):
                            j = s // 128 + jj
                            t = int(chunk_tt[j])
                            oh = OH.tile([128, 128], f32, tag="oh")
                            nc.vector.tensor_scalar(
                                oh[:], iota[:], tloc[:, j:j + 1],
                                dcol[:, j:j + 1],
                                op0=Alu.is_equal, op1=Alu.mult)
                            si = (t - g * GROUP_TT) // SUB_TT
                            ci = (t % SUB_TT) * 128
                            nc.tensor.matmul(
                                ps_tiles[si][:, ci:ci + 128],
                                lhsT=msg[:, jj, :], rhs=oh[:],
                                start=bool(first[j]), stop=bool(last[j]),
                                skip_group_check=True)
                    for si, stts in enumerate(subs):
                        w = len(stts) * 128
                        n0 = stts[0] * 128
                        sup = S.tile([HIDDEN, SUB_TT * 128], f32, tag="sup")
                        nc.vector.tensor_tensor(
                            out=sup[:, :w], in0=ps_tiles[si][:, :w],
                            in1=h0s[:, n0:n0 + w], op=Alu.add)
                        d_ps = PSM.tile([HIDDEN, SUB_TT * 128], f32,
                                        tag="dense")
                        nc.tensor.matmul(
                            d_ps[:, :w],
                            lhsT=wl[:, li * HIDDEN:(li + 1) * HIDDEN],
                            rhs=sup[:, :w], start=True, stop=False)
                        nc.tensor.matmul(
                            d_ps[:, :w],
                            lhsT=il[:, li * HIDDEN:(li + 1) * HIDDEN],
                            rhs=sup[:, :w], start=False, stop=True)
                        ht = S.tile([HIDDEN, SUB_TT * 128], f32, tag="ht")
                        nc.scalar.activation(ht[:, :w], d_ps[:, :w], Act.Relu)
                        if not is_last:
                            for tti, t in enumerate(stts):
                                tp_ps = PSM.tile([128, HIDDEN], f32,
                                                 tag="tp")
                                nc.tensor.transpose(
                                    tp_ps[:],
                                    ht[:, tti * 128:(tti + 1) * 128],
                                    ident[:HIDDEN, :HIDDEN])
                                hs_sb = S.tile([128, HIDDEN], f32, tag="hs")
                                nc.vector.tensor_scalar(
                                    hs_sb[:], tp_ps[:], dinvo[:, t:t + 1],
                                    None, op0=Alu.mult)
                                nc.sync.dma_start(
                                    nxt_shard[t * 128:(t + 1) * 128, :],
                                    hs_sb[:])
                        else:
                            o_ps = PSM.tile([N_CLASSES, SUB_TT * 128], f32,
                                            tag="dense")
                            nc.tensor.matmul(o_ps[:, :w], lhsT=wout[:],
                                             rhs=ht[:, :w],
                                             start=True, stop=True)
                            o_sb = S.tile([N_CLASSES, SUB_TT * 128], f32,
                                          tag="osb")
                            nc.vector.tensor_scalar(
                                o_sb[:, :w], o_ps[:, :w], bout[:], None,
                                op0=Alu.add)
                            for tti, t in enumerate(stts):
                                tp_ps = PSM.tile([128, HIDDEN], f32,
                                                 tag="tp")
                                nc.tensor.transpose(
                                    tp_ps[:, :N_CLASSES],
                                    o_sb[:, tti * 128:(tti + 1) * 128],
                                    ident[:N_CLASSES, :N_CLASSES])
                                ot = S.tile([128, N_CLASSES], f32, tag="ot")
                                nc.vector.tensor_copy(
                                    ot[:], tp_ps[:, :N_CLASSES])
                                nc.sync.dma_start(
                                    out_dram[t * 128:(t + 1) * 128, :],
                                    ot[:])
                if not is_last:
                    nc.gpsimd.collective_compute(
                        "AllGather", Alu.bypass, replica_groups=rg,
                        ins=[nxt_shard.opt()],
                        outs=[hs_full[li + 1].opt()])

    nc.compile()
    return nc


def kernel(x, edge_index, W0, b0, Wl, W_out, b_out):
    from concourse.bass_utils import run_bass_kernel_spmd

    x = np.asarray(x, dtype=np.float32)
    edge_index = np.asarray(edge_index)
    W0 = np.asarray(W0, dtype=np.float32)
    b0 = np.asarray(b0, dtype=np.float32)
    Wl = np.asarray(Wl, dtype=np.float32)
    W_out = np.asarray(W_out, dtype=np.float32)
    b_out = np.asarray(b_out, dtype=np.float32)

    struct, per_core = _preprocess(edge_index)
    nc = _build_program(struct)

    betas = np.array(
        [math.log(LAMDA / (i + 1) + 1.0) for i in range(N_LAYERS)],
        dtype=np.float32)
    wl_host = np.zeros((HIDDEN, N_LAYERS * HIDDEN), np.float32)
    il_host = np.zeros((HIDDEN, N_LAYERS * HIDDEN), np.float32)
    eye = np.eye(HIDDEN, dtype=np.float32)
    for i in range(N_LAYERS):
        wl_host[:, i * HIDDEN:(i + 1) * HIDDEN] = betas[i] * Wl[i]
        il_host[:, i * HIDDEN:(i + 1) * HIDDEN] = (1.0 - betas[i]) * eye
    iota_host = np.tile(np.arange(128, dtype=np.float32), (128, 1))
    ident_host = np.eye(128, dtype=np.float32)

    in_maps = []
    for c in range(NCORES):
        xc = np.zeros((NPC_PAD, N_FEATS), np.float32)
        xc[:NPC] = x[c * NPC:(c + 1) * NPC]
        pc = per_core[c]
        in_maps.append({
            "x": xc,
            "idx": pc["idx"],
            "tloc": pc["tloc"],
            "dcol": pc["dcol"],
            "dinvo": pc["dinvo"],
            "w0": W0,
            "b0": b0.reshape(HIDDEN, 1),
            "wl": wl_host,
            "il": il_host,
            "wout": W_out,
            "bout": b_out.reshape(N_CLASSES, 1),
            "iota": iota_host,
            "ident": ident_host,
        })

    res = run_bass_kernel_spmd(
        nc, in_maps, core_ids=list(range(NCORES)), trace=TRACE)
    _LAST_RESULT["res"] = res
    out = np.concatenate(
        [res.results[c]["out"][:NPC] for c in range(NCORES)], axis=0)
    return out



# revision 2
# speedup vs baseline: 1.2273x; 1.2273x over previous
"""GCNII (nn_GCNII_17626545783193) Bass/Trainium2 kernel, 8 NeuronCores. v2.

Strategy (target-node sharding, feature-major compute, f16 states):
  - Nodes sharded 12500/core (padded to 12544 = 98*128). Edges partitioned
    by target core; self-loops are real edges.
  - Hidden state stored f16 as S_l = dinv * h_l. Gathered in PAIR rows
    (2 nodes = 128 f16 = 256B) so dma_gather's 256B-elem floor is met;
    edge slots are split by source parity so each 128-edge block uses a
    uniform half of the gathered pair row as matmul lhsT.
  - gcn_norm fully factored out of the edge stream: the one-hot is a pure
    0/1 matrix (batched 3D is_equal on DVE via tensor_tensor, which never
    enters 2-port mode -> no SWDGE descgen contention). The 0.9*dinv[t]
    target factor is folded into per-partition post-transpose scales
    (s2 = 0.9*dinv^2 for stored state) and into h0s' = h0/(9*dinv[t]).
  - AllGather per layer is split in two half-shard collectives (f16),
    issued early so they overlap compute: layer compute is split into
    PHASE0 (edges with sources in half a, accumulated into an SBUF
    partial) and PHASE1 (half-b sources + dense update + output).
"""
import sys
sys.path.insert(0, "/opt/trn_rl_repo")

import math
import numpy as np

# ---- problem constants (hardcoded per spec) ----
N_NODES = 100000
N_FEATS = 128
HIDDEN = 64
N_CLASSES = 40
N_LAYERS = 8
ALPHA = 0.1
LAMDA = 0.5
NCORES = 8

NPC = N_NODES // NCORES            # 12500 real nodes per core
TPC = (NPC + 127) // 128           # 98 target tiles per core
NPC_PAD = TPC * 128                # 12544
N_PAD = NPC_PAD * NCORES           # 100352
HALF_TT = TPC // 2                 # 49 target tiles per half
NPC_HALF = HALF_TT * 128           # 6272 nodes per half-shard
NPAIR_HALF = NPC_HALF * NCORES // 2   # 25088 pair rows per half-full
GROUP_TT = 8                       # target tiles per group (sub-aligned)
SUB_TT = 4                         # target tiles per psum tile
AG_A_GROUP = (HALF_TT - 1) // GROUP_TT   # group whose stores finish half a
CALL_MAX = 8192                    # max indices per dma_gather call
OHB = 16                           # one-hot chunks built per DVE op

TRACE = False          # test.py sets this for profiling
_LAST_RESULT = {}      # test.py reads exec_time from here


def _preprocess(edge_index):
    """Build the cross-core-uniform edge stream structure + per-core data.

    Slots are keyed (target tile, source half, source parity); slot sizes
    are the max over cores, rounded to 128, so the instruction stream is
    identical on all cores (SPMD).
    """
    row = np.asarray(edge_index[0], dtype=np.int64)
    col = np.asarray(edge_index[1], dtype=np.int64)
    deg = np.bincount(col, minlength=N_NODES).astype(np.float32) + 1.0
    dinv = (1.0 / np.sqrt(deg)).astype(np.float32)

    loops = np.arange(N_NODES, dtype=np.int64)
    r = np.concatenate([row, loops])
    cl = np.concatenate([col, loops])
    core = cl // NPC
    lcol = cl - core * NPC                       # 0..12499 local target
    tt = lcol >> 7                               # target tile 0..97
    src_core = r // NPC
    src_loc = r - src_core * NPC                 # 0..12499 within shard
    k = (src_loc >= NPC_HALF).astype(np.int64)   # source half 0/1
    # row index within the half-full pair array [NPAIR_HALF, 128]:
    # half-full h layout = concat over cores of shard rows
    # [h*NPC_HALF : h*NPC_HALF+NPC_HALF]
    loc_h = src_loc - k * NPC_HALF               # 0..6271 within half
    gidx = src_core * NPC_HALF + loc_h           # node row in half-full
    pr = gidx >> 1                               # pair row (int16 range)
    par = gidx & 1                               # source parity

    assert NPAIR_HALF <= 32768

    ngroups = -(-TPC // GROUP_TT)                # 13 target groups

    # slot = (tt, k, par)
    slot = ((core * TPC + tt) * 2 + k) * 2 + par
    counts = np.bincount(slot, minlength=NCORES * TPC * 4).reshape(
        NCORES, TPC, 2, 2)
    maxc = counts.max(axis=0)                    # [TPC, 2, 2]
    slot_chunks = -(-maxc // 128)                # ceil; 0 allowed
    slot_len = slot_chunks * 128

    # layout: phase-major (k), then group, then (par, tt)
    slot_off = np.zeros((TPC, 2, 2), np.int64)
    off = 0
    chunk_tt = []
    calls = {}                                   # (k, g) -> [(start, n)]
    for kk in range(2):
        for g in range(ngroups):
            tts = range(g * GROUP_TT, min((g + 1) * GROUP_TT, TPC))
            run_start = off
            for p in range(2):
                for t in tts:
                    slot_off[t, kk, p] = off
                    off += slot_len[t, kk, p]
                    chunk_tt += [t] * slot_chunks[t, kk, p]
            cl_ = []
            s = run_start
            while s < off:
                n = min(CALL_MAX, off - s)
                cl_.append((int(s), int(n)))
                s += n
            calls[(kk, g)] = cl_
    e_pad = off
    nchk = e_pad // 128
    chunk_tt = np.asarray(chunk_tt, np.int64)
    chunk_k = np.zeros(nchk, np.int64)
    # chunk_k: phase of each chunk (phase-major layout -> boundary index)
    kk1_start = int(slot_off[:, 1, :].min()) if slot_len[:, 1, :].sum() else e_pad
    chunk_k[kk1_start // 128:] = 1

    # start/stop per (phase, psum sub): exactly one start (first chunk into
    # the bank) and one stop (last) per accumulation group.
    sub_id = chunk_tt // SUB_TT
    acc_id = chunk_k * 1000 + sub_id
    first = np.zeros(nchk, bool)
    last = np.zeros(nchk, bool)
    for sg in np.unique(acc_id):
        js = np.nonzero(acc_id == sg)[0]
        first[js[0]] = True
        last[js[-1]] = True
    # which (phase, sub) combos have chunks
    nsubs = -(-TPC // SUB_TT)
    has_chunks = np.zeros((2, nsubs), bool)
    for kk in range(2):
        m = chunk_k == kk
        for sg in np.unique(sub_id[m]):
            has_chunks[kk, sg] = True

    per_core = []
    for c in range(NCORES):
        m = core == c
        skey = (tt[m] * 2 + k[m]) * 2 + par[m]
        order = np.argsort(skey, kind="stable")
        sorted_key = skey[order]
        firsts = np.searchsorted(sorted_key, sorted_key, side="left")
        ranks = np.arange(len(sorted_key)) - firsts
        tt_o = tt[m][order]
        k_o = k[m][order]
        p_o = par[m][order]
        pos = slot_off[tt_o, k_o, p_o] + ranks

        idx_stream = np.zeros(e_pad, np.int16)
        tloc_stream = np.full(e_pad, 255.0, np.float16)
        idx_stream[pos] = pr[m][order].astype(np.int16)
        tloc_stream[pos] = (lcol[m][order] & 127).astype(np.float16)

        idx_w = np.tile(
            idx_stream.reshape(e_pad // 16, 16).T, (8, 1)
        ).astype(np.int16)
        tloc_arr = np.ascontiguousarray(tloc_stream.reshape(nchk, 128).T)

        dloc = np.zeros(NPC_PAD, np.float32)
        dloc[:NPC] = dinv[c * NPC:(c + 1) * NPC]
        def col128(v):
            return np.ascontiguousarray(v.reshape(TPC, 128).T)
        per_core.append(dict(
            idx=idx_w, tloc=tloc_arr,
            dinvc=col128(dloc),                          # layer0 S scale
            s2c=col128((0.9 * dloc * dloc).astype(np.float32)),
            c0c=col128(np.where(dloc > 0, 1.0 / (9.0 * dloc), 0.0)
                       .astype(np.float32)),
            drowc=col128((0.9 * dloc).astype(np.float32)),
        ))

    # parity of each 128-chunk (uniform by construction)
    chunk_par = np.zeros(nchk, np.int64)
    for t in range(TPC):
        for kk in range(2):
            for p in range(2):
                o = slot_off[t, kk, p]
                n = slot_len[t, kk, p]
                chunk_par[o // 128:(o + n) // 128] = p

    struct = dict(
        e_pad=e_pad, nchk=nchk, ngroups=ngroups,
        calls=calls, chunk_tt=chunk_tt, chunk_par=chunk_par,
        first=first, last=last, has_chunks=has_chunks,
    )
    return struct, per_core


def _build_program(struct):
    import concourse.bass as bass
    import concourse.mybir as mybir
    import concourse.tile as tile
    import concourse.bacc as bacc

    dt = mybir.dt
    f32 = dt.float32
    f16 = dt.float16
    Alu = mybir.AluOpType
    Act = mybir.ActivationFunctionType

    e_pad = struct["e_pad"]
    nchk = struct["nchk"]
    ngroups = struct["ngroups"]
    calls = struct["calls"]
    chunk_tt = struct["chunk_tt"]
    chunk_par = struct["chunk_par"]
    first = struct["first"]
    last = struct["last"]
    has_chunks = struct["has_chunks"]
    rg = [list(range(NCORES))]
    nsubs = -(-TPC // SUB_TT)

    nc = bacc.Bacc("TRN2", target_bir_lowering=False, debug=False,
                   num_devices=NCORES, num_swdge_queues=4)
    call_ct = [0]

    x_in = nc.dram_tensor("x", [NPC_PAD, N_FEATS], f32, kind="ExternalInput")
    idx_in = nc.dram_tensor("idx", [128, e_pad // 16], dt.int16,
                            kind="ExternalInput")
    tloc_in = nc.dram_tensor("tloc", [128, nchk], f16, kind="ExternalInput")
    dinvc_in = nc.dram_tensor("dinvc", [128, TPC], f32, kind="ExternalInput")
    s2c_in = nc.dram_tensor("s2c", [128, TPC], f32, kind="ExternalInput")
    c0c_in = nc.dram_tensor("c0c", [128, TPC], f32, kind="ExternalInput")
    drowc_in = nc.dram_tensor("drowc", [128, TPC], f32, kind="ExternalInput")
    w0_in = nc.dram_tensor("w0", [N_FEATS, HIDDEN], f32, kind="ExternalInput")
    b0_in = nc.dram_tensor("b0", [HIDDEN, 1], f32, kind="ExternalInput")
    wl_in = nc.dram_tensor("wl", [HIDDEN, N_LAYERS * HIDDEN], f32,
                           kind="ExternalInput")
    il_in = nc.dram_tensor("il", [HIDDEN, N_LAYERS * HIDDEN], f32,
                           kind="ExternalInput")
    wout_in = nc.dram_tensor("wout", [HIDDEN, N_CLASSES], f32,
                             kind="ExternalInput")
    boutb_in = nc.dram_tensor("boutb", [128, N_CLASSES], f32,
                              kind="ExternalInput")
    iota_in = nc.dram_tensor("iota", [128, 128], f16, kind="ExternalInput")
    ident_in = nc.dram_tensor("ident", [128, 128], f32, kind="ExternalInput")
    out_dram = nc.dram_tensor("out", [NPC_PAD, N_CLASSES], f32,
                              kind="ExternalOutput")

    with tile.TileContext(nc) as tc:
        with (
            tc.tile_pool(name="persist", bufs=1) as P,
            tc.tile_pool(name="work", bufs=3) as S,
            tc.tile_pool(name="msgp", bufs=3) as MSG,
            tc.tile_pool(name="ohp", bufs=6) as OH,
            tc.tile_pool(name="idxp", bufs=3) as IDX,
            tc.tile_pool(name="psagg", bufs=4, space="PSUM") as PSA,
            tc.tile_pool(name="psmisc", bufs=2, space="PSUM") as PSM,
            tc.tile_pool(name="dram", bufs=1, space="DRAM") as DR,
        ):
            def pload(name, src, shape, dtype=f32):
                t = P.tile(shape, dtype, tag=name)
                nc.sync.dma_start(t[:], src[:])
                return t

            iota = pload("iota", iota_in, [128, 128], f16)
            ident = pload("ident", ident_in, [128, 128])
            w0 = pload("w0", w0_in, [N_FEATS, HIDDEN])
            b0 = pload("b0", b0_in, [HIDDEN, 1])
            wl = pload("wl", wl_in, [HIDDEN, N_LAYERS * HIDDEN])
            il = pload("il", il_in, [HIDDEN, N_LAYERS * HIDDEN])
            wout = pload("wout", wout_in, [HIDDEN, N_CLASSES])
            boutb = pload("boutb", boutb_in, [128, N_CLASSES])
            tloc = pload("tloc", tloc_in, [128, nchk], f16)
            dinvc = pload("dinvc", dinvc_in, [128, TPC])
            s2c = pload("s2c", s2c_in, [128, TPC])
            c0c = pload("c0c", c0c_in, [128, TPC])
            drowc = pload("drowc", drowc_in, [128, TPC])
            h0s = P.tile([HIDDEN, NPC_PAD], f16, tag="h0s")
            partial = P.tile([HIDDEN, NPC_PAD], f32, tag="partial")

            # per-layer DRAM state: shard halves + gathered halves
            sh_a = [DR.tile([NPC_HALF, HIDDEN], f16, tag=f"sha{j}",
                            name=f"sh_a{j}") for j in range(N_LAYERS)]
            sh_b = [DR.tile([NPC_HALF, HIDDEN], f16, tag=f"shb{j}",
                            name=f"sh_b{j}") for j in range(N_LAYERS)]
            fl_a = [DR.tile([NPAIR_HALF, 128], f16, tag=f"fla{j}",
                            addr_space="Shared", name=f"fl_a{j}")
                    for j in range(N_LAYERS)]
            fl_b = [DR.tile([NPAIR_HALF, 128], f16, tag=f"flb{j}",
                            addr_space="Shared", name=f"fl_b{j}")
                    for j in range(N_LAYERS)]

            def store_state(li, t, src_sb):
                """Write node-major f16 state tile to the right half-shard."""
                if t < HALF_TT:
                    dst, r0 = sh_a[li], t * 128
                else:
                    dst, r0 = sh_b[li], (t - HALF_TT) * 128
                nc.sync.dma_start(dst[r0:r0 + 128, :], src_sb[:])

            def issue_ag(li, half):
                if half == 0:
                    nc.gpsimd.collective_compute(
                        "AllGather", Alu.bypass, replica_groups=rg,
                        ins=[sh_a[li].opt()], outs=[fl_a[li].opt()])
                else:
                    nc.gpsimd.collective_compute(
                        "AllGather", Alu.bypass, replica_groups=rg,
                        ins=[sh_b[li].opt()], outs=[fl_b[li].opt()])

            # ---------------- layer 0: h0 = relu(x @ W0 + b0) --------------
            for t in range(TPC):
                xt = S.tile([128, N_FEATS], f32, tag="xt")
                nc.sync.dma_start(xt[:], x_in[t * 128:(t + 1) * 128, :])
                xT_ps = PSM.tile([N_FEATS, 128], f32, tag="tp")
                nc.tensor.transpose(xT_ps[:], xt[:], ident[:])
                xT = S.tile([N_FEATS, 128], f32, tag="xT")
                nc.scalar.activation(xT[:], xT_ps[:], Act.Copy)
                h_ps = PSM.tile([HIDDEN, 128], f32, tag="dense")
                nc.tensor.matmul(h_ps[:], lhsT=w0[:], rhs=xT[:],
                                 start=True, stop=True)
                h0t = S.tile([HIDDEN, 128], f32, tag="h0t")
                nc.scalar.activation(h0t[:], h_ps[:], Act.Relu, bias=b0[:])
                # node-major h0 for the two per-column scales
                tp_ps = PSM.tile([128, HIDDEN], f32, tag="tp")
                nc.tensor.transpose(tp_ps[:], h0t[:],
                                    ident[:HIDDEN, :HIDDEN])
                hs_sb = S.tile([128, HIDDEN], f16, tag="hs")
                nc.scalar.activation(hs_sb[:], tp_ps[:], Act.Copy,
                                     scale=dinvc[:, t:t + 1])
                store_state(0, t, hs_sb)
                h0n = S.tile([128, HIDDEN], f32, tag="h0n")
                nc.scalar.activation(h0n[:], tp_ps[:], Act.Copy,
                                     scale=c0c[:, t:t + 1])
                h0T_ps = PSM.tile([HIDDEN, 128], f32, tag="tp")
                nc.tensor.transpose(h0T_ps[:], h0n[:], ident[:])
                nc.scalar.activation(h0s[:, t * 128:(t + 1) * 128],
                                     h0T_ps[:], Act.Copy)
                if t == HALF_TT - 1:
                    issue_ag(0, 0)
            issue_ag(0, 1)

            # ---------------- GCNII layers ----------------
            for li in range(N_LAYERS):
                is_last = li == N_LAYERS - 1
                for kk in range(2):                     # phase = source half
                    cur = (fl_a if kk == 0 else fl_b)[li]
                    for g in range(ngroups):
                        tts = list(range(g * GROUP_TT,
                                         min((g + 1) * GROUP_TT, TPC)))
                        subs = {}
                        for t in tts:
                            subs.setdefault(t // SUB_TT, []).append(t)
                        ps_tiles = {}
                        for sg, stts in subs.items():
                            if has_chunks[kk, sg]:
                                ps_tiles[sg] = PSA.tile(
                                    [HIDDEN, SUB_TT * 128], f32, tag="agg",
                                    name=f"agg{li}_{kk}_{g}_{sg}")
                        for (s, n) in calls[(kk, g)]:
                            idx_t = IDX.tile([128, n // 16], dt.int16,
                                             tag="idxs")
                            nc.sync.dma_start(
                                idx_t[:], idx_in[:, s // 16:(s + n) // 16])
                            msg = MSG.tile([128, CALL_MAX // 128, 128], f16,
                                           tag="msg")
                            nc.gpsimd.dma_gather(
                                msg[:, :n // 128, :], cur[:],
                                idx_t[:], num_idxs=n, num_idxs_reg=n,
                                elem_size=128, single_packet=False,
                                queue_num=call_ct[0] % 4)
                            call_ct[0] += 1
                            nblk = n // 128
                            for j0 in range(0, nblk, OHB):
                                nb = min(OHB, nblk - j0)
                                oh = OH.tile([128, OHB, 128], f16, tag="oh")
                                jj0 = s // 128 + j0
                                iota_b = iota[:].unsqueeze(1).broadcast_to(
                                    [128, nb, 128])
                                tloc_b = tloc[:, jj0:jj0 + nb].unsqueeze(
                                    2).broadcast_to([128, nb, 128])
                                nc.vector.tensor_tensor(
                                    out=oh[:, :nb, :], in0=iota_b,
                                    in1=tloc_b, op=Alu.is_equal)
                                for j in range(nb):
                                    gj = jj0 + j
                                    t = int(chunk_tt[gj])
                                    p = int(chunk_par[gj])
                                    sg = t // SUB_TT
                                    ci = (t % SUB_TT) * 128
                                    nc.tensor.matmul(
                                        ps_tiles[sg][:, ci:ci + 128],
                                        lhsT=msg[:, j0 + j,
                                                 p * 64:p * 64 + 64],
                                        rhs=oh[:, j, :],
                                        start=bool(first[gj]),
                                        stop=bool(last[gj]),
                                        skip_group_check=True)
                        if kk == 0:
                            # phase0: flush psum (+h0s) into partial
                            for sg, stts in subs.items():
                                w = len(stts) * 128
                                n0 = stts[0] * 128
                                if sg in ps_tiles:
                                    nc.vector.tensor_tensor(
                                        out=partial[:, n0:n0 + w],
                                        in0=ps_tiles[sg][:, :w],
                                        in1=h0s[:, n0:n0 + w], op=Alu.add)
                                else:
                                    nc.scalar.activation(
                                        partial[:, n0:n0 + w],
                                        h0s[:, n0:n0 + w], Act.Copy)
                            continue
                        # phase1: finish support, dense update, output
                        for sg, stts in subs.items():
                            w = len(stts) * 128
                            n0 = stts[0] * 128
                            if sg in ps_tiles:
                                sup = S.tile([HIDDEN, SUB_TT * 128], f32,
                                             tag="sup")
                                nc.vector.tensor_tensor(
                                    out=sup[:, :w], in0=ps_tiles[sg][:, :w],
                                    in1=partial[:, n0:n0 + w], op=Alu.add)
                                sup_ap = sup[:, :w]
                            else:
                                sup_ap = partial[:, n0:n0 + w]
                            d_ps = PSM.tile([HIDDEN, SUB_TT * 128], f32,
                                            tag="dense")
                            nc.tensor.matmul(
                                d_ps[:, :w],
                                lhsT=wl[:, li * HIDDEN:(li + 1) * HIDDEN],
                                rhs=sup_ap, start=True, stop=False)
                            nc.tensor.matmul(
                                d_ps[:, :w],
                                lhsT=il[:, li * HIDDEN:(li + 1) * HIDDEN],
                                rhs=sup_ap, start=False, stop=True)
                            ht = S.tile([HIDDEN, SUB_TT * 128], f32,
                                        tag="ht")
                            nc.scalar.activation(ht[:, :w], d_ps[:, :w],
                                                 Act.Relu)
                            if not is_last:
                                for tti, t in enumerate(stts):
                                    tp_ps = PSM.tile([128, HIDDEN], f32,
                                                     tag="tp")
                                    nc.tensor.transpose(
                                        tp_ps[:],
                                        ht[:, tti * 128:(tti + 1) * 128],
                                        ident[:HIDDEN, :HIDDEN])
                                    hs_sb = S.tile([128, HIDDEN], f16,
                                                   tag="hs")
                                    nc.scalar.activation(
                                        hs_sb[:], tp_ps[:], Act.Copy,
                                        scale=s2c[:, t:t + 1])
                                    store_state(li + 1, t, hs_sb)
                            else:
                                o_ps = PSM.tile([N_CLASSES, SUB_TT * 128],
                                                f32, tag="dense")
                                nc.tensor.matmul(o_ps[:, :w], lhsT=wout[:],
                                                 rhs=ht[:, :w],
                                                 start=True, stop=True)
                                o_sb = S.tile([N_CLASSES, SUB_TT * 128],
                                              f32, tag="osb")
                                nc.scalar.activation(o_sb[:, :w],
                                                     o_ps[:, :w], Act.Copy)
                                for tti, t in enumerate(stts):
                                    tp_ps = PSM.tile([128, HIDDEN], f32,
                                                     tag="tp")
                                    nc.tensor.transpose(
                                        tp_ps[:, :N_CLASSES],
                                        o_sb[:, tti * 128:(tti + 1) * 128],
                                        ident[:N_CLASSES, :N_CLASSES])
                                    ot = S.tile([128, N_CLASSES], f32,
                                                tag="ot")
                                    nc.vector.scalar_tensor_tensor(
                                        out=ot[:], in0=tp_ps[:, :N_CLASSES],
                                        scalar=drowc[:, t:t + 1],
                                        in1=boutb[:], op0=Alu.mult,
                                        op1=Alu.add)
                                    nc.sync.dma_start(
                                        out_dram[t * 128:(t + 1) * 128, :],
                                        ot[:])
                        if not is_last:
                            if g == AG_A_GROUP:
                                issue_ag(li + 1, 0)
                            elif g == ngroups - 1:
                                issue_ag(li + 1, 1)

    nc.compile()
    return nc


def prepare(x, edge_index, W0, b0, Wl, W_out, b_out):
    x = np.asarray(x, dtype=np.float32)
    edge_index = np.asarray(edge_index)
    W0 = np.asarray(W0, dtype=np.float32)
    b0 = np.asarray(b0, dtype=np.float32)
    Wl = np.asarray(Wl, dtype=np.float32)
    W_out = np.asarray(W_out, dtype=np.float32)
    b_out = np.asarray(b_out, dtype=np.float32)

    struct, per_core = _preprocess(edge_index)
    nc = _build_program(struct)

    betas = np.array(
        [math.log(LAMDA / (i + 1) + 1.0) for i in range(N_LAYERS)],
        dtype=np.float32)
    wl_host = np.zeros((HIDDEN, N_LAYERS * HIDDEN), np.float32)
    il_host = np.zeros((HIDDEN, N_LAYERS * HIDDEN), np.float32)
    eye = np.eye(HIDDEN, dtype=np.float32)
    for i in range(N_LAYERS):
        wl_host[:, i * HIDDEN:(i + 1) * HIDDEN] = betas[i] * Wl[i]
        il_host[:, i * HIDDEN:(i + 1) * HIDDEN] = (1.0 - betas[i]) * eye
    iota_host = np.tile(np.arange(128, dtype=np.float16), (128, 1))
    ident_host = np.eye(128, dtype=np.float32)
    boutb_host = np.tile(b_out.reshape(1, N_CLASSES), (128, 1)).astype(
        np.float32)

    in_maps = []
    for c in range(NCORES):
        xc = np.zeros((NPC_PAD, N_FEATS), np.float32)
        xc[:NPC] = x[c * NPC:(c + 1) * NPC]
        pc = per_core[c]
        in_maps.append({
            "x": xc,
            "idx": pc["idx"],
            "tloc": pc["tloc"],
            "dinvc": pc["dinvc"],
            "s2c": pc["s2c"],
            "c0c": pc["c0c"],
            "drowc": pc["drowc"],
            "w0": W0,
            "b0": b0.reshape(HIDDEN, 1),
            "wl": wl_host,
            "il": il_host,
            "wout": W_out,
            "boutb": boutb_host,
            "iota": iota_host,
            "ident": ident_host,
        })

    return nc, in_maps


def kernel(x, edge_index, W0, b0, Wl, W_out, b_out):
    from concourse.bass_utils import run_bass_kernel_spmd

    nc, in_maps = prepare(x, edge_index, W0, b0, Wl, W_out, b_out)
    res = run_bass_kernel_spmd(
        nc, in_maps, core_ids=list(range(NCORES)), trace=TRACE)
    _LAST_RESULT["res"] = res
    out = np.concatenate(
        [res.results[c]["out"][:NPC] for c in range(NCORES)], axis=0)
    return out
